# revision 35
# baseline (speedup 1.0000x reference)
"""Distributed Bass kernel for attention-energy softmax on 8 TRN2 NeuronCores.

Computes: softmax(enc @ W.T @ h + (b.h)) == softmax(enc @ (W.T @ h)) over S=32768.
The bias term b.h is a constant shift across all energies and cancels in
softmax, so b is unused.

Sharding (flash-softmax style): encoder_output split along S into 8 shards of
4096 rows; each shard is host-transposed to [H, S_shard] and cast to fp16 so
the contraction dim (H) lands on SBUF partitions and DMA/TensorE run at 16-bit
rates. W and h are replicated fp16. fp16 products accumulate exactly in fp32
PSUM; softmax rel err of the fp16 path is ~6e-3 (measured) vs the 2e-2 gate.

Per core (no cross-core sync -> per-core exec time is independent of the
runtime's multi-core dispatch stagger):
  v_row[1,1024] = h-chunk-stationary @ Wh (moving, N=512)   16 matmuls
  v_col[128,8]  = per-chunk PE transpose of v_row (outer product with [1,1])
  e[4x1024]     = sum_hc vh_col[:,hc].T @ enc_slab_hc        64 matmuls (M=1,
                  N=512) into PSUM rows {0,32,64,96} x 2 banks (legal
                  tile_position col values), so stats run at FD=1024 with
                  native per-partition bias
  row stats: one reduce_max + one Exp with accum_out -> exp(e - m_r), (m_r, s_r)
  outputs: exp slices [4,1024] + stats [4,2]; the host gather/unshard applies
  the global softmax normalization (max/sum combine over 32 scalars and one
  rescale per shard), as hinted (distributed softmax with max/sum reduction).
"""

import sys

sys.path.insert(0, "/opt/trn_rl_repo")

import numpy as np

import concourse.bacc as bacc
import concourse.mybir as mybir
import concourse.tile as tile
from concourse.bass_utils import run_bass_kernel_spmd

N_CORES = 8
H = 1024
S = 32768
S_SHARD = S // N_CORES          # 4096
HC = H // 128                   # 8 h-chunks of 128 (contraction tiles)
NR = 4                          # PSUM partition rows (0,32,64,96)
RW = S_SHARD // NR              # 1024 energies per row (2 PSUM banks)
FP32 = mybir.dt.float32
FP16 = mybir.dt.float16

_compiled_nc = None


def _build():
    nc = bacc.Bacc(
        "TRN2", target_bir_lowering=False, debug=False, num_devices=N_CORES
    )

    encT = nc.dram_tensor("encT", [H, S_SHARD], FP16, kind="ExternalInput")
    hh2 = nc.dram_tensor("hh2", [128, HC], FP16, kind="ExternalInput")
    # W packed by j-halves: Wp[half, k, j'] = W[k, half*512 + j']
    Wp = nc.dram_tensor("Wp", [2, H, H // 2], FP16, kind="ExternalInput")
    # per row: 1024 exp values, then (m_row, s_row)
    out_ext = nc.dram_tensor("out", [NR, RW + 2], FP32, kind="ExternalOutput")

    EXP = mybir.ActivationFunctionType.Exp
    AX = mybir.AxisListType.X

    with tile.TileContext(nc) as tc:
        with (
            tc.tile_pool(name="sb", bufs=1) as sb,
            tc.tile_pool(name="enc", bufs=5) as encp,
        ):
            # --- small inputs / constants ---
            hh_sb = sb.tile([128, HC], FP16, tag="hh")
            one1 = sb.tile([1, 1], FP32, tag="one1")
            W_half = [
                sb.tile([128, HC * 512], FP16, tag=f"W{j}", name=f"W{j}")
                for j in range(2)
            ]

            nc.sync.dma_start(out=hh_sb[:, :], in_=hh2[:, :])
            for j in range(2):
                nc.sync.dma_start(
                    out=W_half[j][:, :].rearrange("p (c j) -> p c j", c=HC),
                    in_=Wp[j, :, :].rearrange("(c p) j -> p c j", p=128),
                )
            nc.vector.memset(one1[:, :], 1.0)
            # touch Exp early so the ACT table load is off the critical path
            warm = sb.tile([1, 1], FP32, tag="warm")
            nc.scalar.activation(warm[0:1, :], one1[0:1, :], EXP)

            # --- v phase, pipelined per j-half ---
            # v_row[0, j] = v[j] = sum_k W[k, j] h[k]
            vrow_half = [
                sb.tile([1, 512], FP32, tag=f"vr{j}", name=f"vr{j}")
                for j in range(2)
            ]
            vcol_half = [
                sb.tile([128, HC // 2], FP16, tag=f"vc{j}", name=f"vc{j}")
                for j in range(2)
            ]
            _ps_cm = tc.tile_pool(name="ps", bufs=1, space="PSUM")
            psp = _ps_cm.__enter__()  # v and e PSUM coexist (6 of 8 banks)
            for j in range(2):
                vr_ps = psp.tile(
                    [1, 512], FP32, tag=f"vrps{j}", name=f"vrps{j}"
                )
                for kc in range(HC):
                    nc.tensor.matmul(
                        vr_ps[0:1, :],
                        lhsT=hh_sb[:, kc : kc + 1],
                        rhs=W_half[j][:, kc * 512 : (kc + 1) * 512],
                        start=(kc == 0),
                        stop=(kc == HC - 1),
                    )
                nc.vector.tensor_copy(vrow_half[j][:, :], vr_ps[0:1, :])
                vc_ps = psp.tile(
                    [128, HC // 2], FP32, tag=f"vcps{j}", name=f"vcps{j}"
                )
                for q in range(HC // 2):
                    nc.tensor.matmul(
                        vc_ps[:, q : q + 1],
                        lhsT=vrow_half[j][0:1, q * 128 : (q + 1) * 128],
                        rhs=one1[0:1, 0:1],
                        start=True,
                        stop=True,
                    )
                # fp16 cast; vcol_half[j][:, q] = v[j*512 + q*128 + p]
                nc.vector.tensor_copy(vcol_half[j][:, :], vc_ps[:, :])

            def vh_slice(hc):  # v chunk hc as a [128, 1] fp16 column
                return vcol_half[hc // (HC // 2)][
                    :, hc % (HC // 2) : hc % (HC // 2) + 1
                ]

            # --- e phase: PSUM [128, 1024]; energies live on rows 0/32/64/96,
            # slice b (512 wide) at (row 32*(b//2), bank b%2) ---
            mx = sb.tile([128, 1], FP32, tag="mx")
            ngx = sb.tile([128, 1], FP32, tag="ngx")
            scratch = sb.tile([128, RW + 2], FP32, tag="scr")
            e_ps = psp.tile([128, RW], FP32, tag="eps")
            nc.vector.memset(e_ps[:, :], 0.0)  # keep unused rows finite
            for hp in range(HC // 2):  # 2 h-chunks per 2 MiB slab
                slab = encp.tile([128, 2 * S_SHARD], FP16, tag="slab")
                nc.sync.dma_start(
                    out=slab[:, :].rearrange("p (c s) -> p c s", c=2),
                    in_=encT[hp * 256 : (hp + 1) * 256, :].rearrange(
                        "(c p) s -> p c s", p=128
                    ),
                )
                for ci in range(2):
                    hc = hp * 2 + ci
                    for b in range(S_SHARD // 512):
                        row = 32 * (b // 2)
                        jb = b % 2
                        nc.tensor.matmul(
                            e_ps[row : row + 1, jb * 512 : (jb + 1) * 512],
                            lhsT=vh_slice(hc),
                            rhs=slab[
                                :,
                                ci * S_SHARD
                                + b * 512 : ci * S_SHARD
                                + (b + 1) * 512,
                            ],
                            start=(hc == 0),
                            stop=(hc == HC - 1),
                            tile_position=(0, row),
                        )
            # per-row stats: exp(e - m_row) + row sums, FD=1024
            nc.vector.reduce_max(mx[:, :], e_ps[:, :], axis=AX)
            nc.vector.tensor_scalar_mul(ngx[:, :], mx[:, :], -1.0)
            nc.scalar.activation(
                scratch[:, 0:RW], e_ps[:, :], EXP,
                bias=ngx[:, :], scale=1.0,
                accum_out=scratch[:, RW + 1 : RW + 2],
            )
            _ps_cm.__exit__(None, None, None)

            nc.vector.tensor_copy(scratch[:, RW : RW + 1], mx[:, :])
            # rows 0/32/64/96 carry the payload: [1024 exp vals, m, s] each
            nc.sync.dma_start(
                out=out_ext[:, :], in_=scratch[0 : 3 * 32 + 1 : 32, :]
            )

    nc.compile()
    return nc


def get_nc():
    global _compiled_nc
    if _compiled_nc is None:
        _compiled_nc = _build()
    return _compiled_nc


def make_in_maps(hidden_state, encoder_output, W):
    h = np.asarray(hidden_state, dtype=np.float32).reshape(H)
    enc = np.asarray(encoder_output, dtype=np.float32).reshape(S, H)
    Wf = np.asarray(W, dtype=np.float32).reshape(H, H)

    h2 = h.reshape(HC, 128).T  # h2[p, c] = h[c*128 + p]
    hh2 = np.ascontiguousarray(h2.astype(np.float16))
    W16 = Wf.astype(np.float16)
    Wp = np.ascontiguousarray(
        np.stack([W16[:, 0:512], W16[:, 512:1024]])
    )  # [2, 1024, 512]

    in_maps = []
    for c in range(N_CORES):
        shard = np.ascontiguousarray(
            enc[c * S_SHARD : (c + 1) * S_SHARD, :].T.astype(np.float16)
        )  # [H, S_SHARD] fp16
        in_maps.append({"encT": shard, "hh2": hh2, "Wp": Wp})
    return in_maps


def unshard(results):
    # gather + global softmax normalization over the 8x4 (max, sum) stats
    payload = np.stack(
        [results[c]["out"].reshape(NR, RW + 2) for c in range(N_CORES)]
    )  # [8, 4, 1026]
    M = payload[:, :, RW].max()
    z = np.exp(payload[:, :, RW] - M)          # [8, 4]
    Z = float((payload[:, :, RW + 1] * z).sum())
    out = np.empty((1, S), dtype=np.float32)
    for c in range(N_CORES):
        vals = payload[c, :, 0:RW] * (z[c] / Z)[:, None]
        out[0, c * S_SHARD : (c + 1) * S_SHARD] = vals.reshape(S_SHARD)
    return out


def kernel(hidden_state, encoder_output, W, b=None, **_unused):
    nc = get_nc()
    in_maps = make_in_maps(hidden_state, encoder_output, W)
    res = run_bass_kernel_spmd(nc, in_maps, core_ids=list(range(N_CORES)))
    return unshard(res.results)
